# revision 15
# baseline (speedup 1.0000x reference)
"""Trainium2 Bass kernel for the pairwise+triplewise cycle-consistency loss.

Strategy (8 NeuronCores, tensor-parallel over rows of each [N,N] block):
  - All six cycle-term matrices have the form  A = U @ nf_j^T  with
    U = nf_i (pairs) or U = nf_i @ G_k (triples), G_k = nf_k^T nf_k [D,D].
    This collapses the [N,N]@[N,N] triple products into [D,D] Gram matmuls.
  - Each core owns a 512-row block R_c. Per term it computes A[R_c,:] and
    A^T[R_c,:] directly (bf16 matmuls, f32 PSUM), row-softmaxes both,
    AllGathers the normalized S21_hat (fp8e4, x64 scaled), and computes M^T
    column-tiles (S12_hat @ S21_hat)^T[jtile, R_c] with fp8 DoubleRow
    matmuls, accumulating rowmax/colmax/diag stats on the fly. The scalar
    loss is assembled on host.
  - Grams are computed as per-core partials and AllReduced in bf16. The
    gram work is placed after term 0's AllGather kick so the AllReduces
    don't delay the first AllGather on the single CC stream.
"""
import sys
sys.path.insert(0, "/opt/trn_rl_repo")

import math
import numpy as np
import ml_dtypes

import concourse.bass as bass
import concourse.mybir as mybir
import concourse.tile as tile
from concourse import bacc
from concourse.bass_utils import run_bass_kernel_spmd
from concourse.masks import make_identity

F32 = mybir.dt.float32
BF16 = mybir.dt.bfloat16
F8 = mybir.dt.float8e4
AX = mybir.AxisListType
OP = mybir.AluOpType
ACT = mybir.ActivationFunctionType
DR = mybir.MatmulPerfMode.DoubleRow

NTOK = 4096          # rows per view
D = 1024             # feature dim
NC = 8               # cores
RPC = NTOK // NC     # rows per core (512)
P = 128
NRT = RPC // P       # rowtiles per core (4)
NS = 8               # 512-col strips of A
DKB = D // P         # d-blocks (8)
NKB = NTOK // P      # k-blocks of the M product (32)
NJG = 8              # jtile groups (4 jtiles each) in M product
SCALE = math.log(NTOK) / 0.1
MARGIN = 0.5
QS = 64.0            # fp8 quant scale for softmax probs (M scaled by QS*QS)

# term table: (is_tri, gram_idx, lhsA, rhsA, lhsAT, rhsAT); lhs names index x_i,
# rhs names index f_i. For tri terms lhs is G[gram_idx] @ x_i.
TERMS = [
    (False, None, 0, 1, 1, 0),   # S01
    (False, None, 0, 2, 2, 0),   # S02
    (False, None, 1, 2, 2, 1),   # S12
    (True, 2, 0, 1, 1, 0),       # S02 @ S21 = nf0 G2 nf1^T
    (True, 1, 0, 2, 2, 0),       # S01 @ S12 = nf0 G1 nf2^T
    (True, 0, 1, 2, 2, 1),       # S10 @ S02 = nf1 G0 nf2^T
]

OUT_W = RPC + 32 + NRT   # racc 512 | colmax32 32 | diag 4


def build_program():
    nc = bacc.Bacc("TRN2", target_bir_lowering=False, debug=False, num_devices=NC)

    xs = [nc.dram_tensor(f"x{i}", [D, RPC], BF16, kind="ExternalInput") for i in range(3)]
    ws = [nc.dram_tensor(f"w{i}", [RPC, D], BF16, kind="ExternalInput") for i in range(3)]
    fs = [nc.dram_tensor(f"f{i}", [D, NTOK], BF16, kind="ExternalInput") for i in range(3)]
    wsel_in = nc.dram_tensor("wsel", [P, P], F32, kind="ExternalInput")
    out = nc.dram_tensor("out", [6, P, OUT_W], F32, kind="ExternalOutput")

    with tile.TileContext(nc) as tc:
        with (
            tc.tile_pool(name="cst", bufs=1) as cst,
            tc.tile_pool(name="lhs", bufs=2) as lhsp,
            tc.tile_pool(name="rhs", bufs=2) as rhsp,
            tc.tile_pool(name="aq", bufs=6) as aqp,
            tc.tile_pool(name="pt", bufs=1) as ptp,
            tc.tile_pool(name="qbf", bufs=2) as qbfp,
            tc.tile_pool(name="qsb", bufs=4) as qsbp,
            tc.tile_pool(name="st", bufs=3) as stp,
            tc.tile_pool(name="sm", bufs=6) as smp,
            tc.tile_pool(name="psB", bufs=6, space="PSUM") as psB,
            tc.tile_pool(name="psT", bufs=2, space="PSUM") as psT,
            tc.tile_pool(name="dram", bufs=1, space="DRAM") as dram,
            tc.tile_pool(name="dram2", bufs=2, space="DRAM") as dram2,
        ):
            # constants
            ident = cst.tile([P, P], F32)
            make_identity(nc, ident)
            identb = cst.tile([P, P], BF16)
            nc.vector.tensor_copy(identb[:], ident[:])
            wsel = cst.tile([P, P], F32)
            nc.sync.dma_start(wsel[:], wsel_in[:])
            nwsel = cst.tile([P, P], F32)
            nc.vector.tensor_scalar_mul(nwsel[:], wsel[:], -1.0)
            nwselBIG = cst.tile([P, P], F32)
            nc.vector.tensor_scalar_mul(nwselBIG[:], wsel[:], -1.0e30)
            # imask4[p, 128b+p] = 1 for b in 0..3 (diag candidate positions)
            imask4 = cst.tile([P, NRT, P], F32)
            for b in range(NRT):
                nc.vector.tensor_copy(imask4[:, b, :], ident[:])

            gins = [dram.tile([D, D], BF16, tag=f"gin{k}", name=f"gin{k}")
                    for k in range(3)]
            gouts = [dram.tile([D, D], BF16, tag=f"gout{k}", addr_space="Shared",
                               name=f"gout{k}") for k in range(3)]

            def compute_gram_partials():
                for k in range(3):
                    w_sb = aqp.tile([P, NRT, D], BF16, tag="aq", name=f"w_sb{k}")
                    nc.sync.dma_start(w_sb[:], ws[k].rearrange("(o p) d -> p o d", p=P))
                    for d1 in range(DKB):
                        for d2 in range(2):
                            ps = psB.tile([P, 512], F32, tag="ps", name=f"gps{k}_{d1}_{d2}")
                            for nt in range(NRT):
                                nc.tensor.matmul(
                                    ps[:], w_sb[:, nt, d1 * P:(d1 + 1) * P],
                                    w_sb[:, nt, d2 * 512:(d2 + 1) * 512],
                                    start=(nt == 0), stop=(nt == NRT - 1))
                            gtmp = stp.tile([P, 512], BF16, tag="gtmp", name=f"gt{k}_{d1}_{d2}")
                            nc.vector.tensor_copy(gtmp[:], ps[:])
                            nc.sync.dma_start(
                                gins[k][d1 * P:(d1 + 1) * P,
                                        d2 * 512:(d2 + 1) * 512], gtmp[:])

            def kick_gram_ar(k):
                nc.gpsimd.collective_compute(
                    "AllReduce", OP.add, replica_groups=[list(range(NC))],
                    ins=[gins[k][:]], outs=[gouts[k][:]])

            # ---------------- helpers ----------------
            def load_x(i, nm):
                t = lhsp.tile([P, DKB, RPC], BF16, tag="lhs", name=f"x_{nm}")
                nc.sync.dma_start(t[:], xs[i].rearrange("(o p) r -> p o r", p=P))
                return t

            def compute_ut(gk, i, nm):
                """U^T[:, R_c] = G_k @ x_i  -> [128, DKB, RPC] bf16 tile."""
                x_sb = load_x(i, f"utx_{nm}")
                ut = lhsp.tile([P, DKB, RPC], BF16, tag="lhs", name=f"ut_{nm}")
                for grp in range(2):
                    pss = [psB.tile([P, 512], F32, tag="ps", name=f"utps_{nm}_{grp}_{d4}")
                           for d4 in range(4)]
                    for half in range(2):
                        gh = rhsp.tile([P, 4, D], BF16, tag="rhs", name=f"gh_{nm}_{grp}_{half}")
                        nc.sync.dma_start(
                            gh[:], gouts[gk][half * 512:(half + 1) * 512]
                            .rearrange("(o p) d -> p o d", p=P))
                        for d4 in range(4):
                            dp = 4 * grp + d4
                            for db in range(4):
                                nc.tensor.matmul(
                                    pss[d4][:], gh[:, db, dp * P:(dp + 1) * P],
                                    x_sb[:, 4 * half + db, :],
                                    start=(half == 0 and db == 0),
                                    stop=(half == 1 and db == 3))
                    for d4 in range(4):
                        nc.scalar.copy(ut[:, 4 * grp + d4, :], pss[d4][:])
                return ut

            def side_chunk(lhs_t, fj, nm, cb=None):
                """A-side chunk [RPC, 4096] raw logits (pre-scale) in 4 quarter tiles.

                cb(s), if given, is invoked after each 512-col strip's matmuls;
                used to interleave the previous term's M-phase jg groups."""
                chunk = [aqp.tile([P, NTOK], F32, tag="aq", name=f"ch_{nm}_{rt}")
                         for rt in range(NRT)]
                for s in range(NS):
                    rsb = rhsp.tile([P, DKB, 512], BF16, tag="rhs", name=f"rs_{nm}_{s}")
                    nc.sync.dma_start(
                        rsb[:], fs[fj][:, s * 512:(s + 1) * 512]
                        .rearrange("(o p) n -> p o n", p=P))
                    for rt in range(NRT):
                        ps = psB.tile([P, 512], F32, tag="ps", name=f"aps_{nm}_{s}_{rt}")
                        for kb in range(DKB):
                            nc.tensor.matmul(
                                ps[:], lhs_t[:, kb, rt * P:(rt + 1) * P],
                                rsb[:, kb, :], start=(kb == 0), stop=(kb == DKB - 1))
                        nc.scalar.copy(chunk[rt][:, s * 512:(s + 1) * 512], ps[:])
                    if cb is not None:
                        cb(s)
                return chunk

            def softmax_quarter(q, nm):
                """in-place exp(SCALE*(x - rowmax)); returns QS/rowsum [P,1]."""
                rm = smp.tile([P, 1], F32, tag="sm", name=f"rm_{nm}")
                nc.vector.reduce_max(rm[:], q[:], axis=AX.X)
                bias = smp.tile([P, 1], F32, tag="sm", name=f"bias_{nm}")
                nc.vector.tensor_scalar_mul(bias[:], rm[:], -SCALE)
                ssum = smp.tile([P, 1], F32, tag="sm", name=f"ss_{nm}")
                nc.scalar.activation(q[:], q[:], ACT.Exp, bias=bias[:], scale=SCALE,
                                     accum_out=ssum[:])
                rs = smp.tile([P, 1], F32, tag="sm", name=f"rs_{nm}")
                nc.vector.reciprocal(rs[:], ssum[:])
                rsq = smp.tile([P, 1], F32, tag="sm", name=f"rsq_{nm}")
                nc.vector.tensor_scalar_mul(rsq[:], rs[:], QS)
                return rsq

            def at_phase(t, lhs_t, fj):
                """A^T side: softmax rows -> normalized fp8 (xQS) -> allgather kick."""
                ag_in = dram2.tile([RPC, NTOK], F8, tag="agin", name=f"agin{t}")
                ag_out = dram2.tile([NTOK, NTOK], F8, tag="agout",
                                    addr_space="Shared", name=f"agout{t}")
                chunk = side_chunk(lhs_t, fj, f"at{t}")
                for rt in range(NRT):
                    rq = softmax_quarter(chunk[rt], f"at{t}_{rt}")
                    qb = qbfp.tile([P, NTOK], F8, tag="qbf", name=f"qb{t}_{rt}")
                    nc.vector.tensor_scalar_mul(qb[:], chunk[rt][:], rq[:])
                    nc.sync.dma_start(ag_in[rt * P:(rt + 1) * P, :], qb[:])
                nc.gpsimd.collective_compute(
                    "AllGather", OP.bypass, replica_groups=[list(range(NC))],
                    ins=[ag_in[:]], outs=[ag_out[:]])
                return ag_out

            def a_phase(t, lhs_t, fj, cb=None):
                """A side: softmax, normalize to fp8 (xQS), transpose into PT tile."""
                pt = ptp.tile([P, NKB, RPC], F8, tag="pt", name=f"pt{t}")
                chunk = side_chunk(lhs_t, fj, f"a{t}", cb=cb)
                for rt in range(NRT):
                    rp = softmax_quarter(chunk[rt], f"a{t}_{rt}")
                    qa = qbfp.tile([P, NTOK], BF16, tag="qbf", name=f"qa{t}_{rt}")
                    nc.vector.tensor_scalar_mul(qa[:], chunk[rt][:], rp[:])
                    for j in range(NKB):
                        tp = psT.tile([P, P], BF16, tag="psT", name=f"tp{t}_{rt}_{j}")
                        nc.tensor.transpose(tp[:], qa[:, j * P:(j + 1) * P],
                                            identb[:])
                        nc.vector.tensor_copy(pt[:, j, rt * P:(rt + 1) * P], tp[:])
                return pt

            def m_phase_start(u):
                """Alloc/zero the stat accumulators for term u's M-phase."""
                racc = stp.tile([P, RPC], F32, tag="racc", name=f"racc{u}")
                nc.vector.memset(racc[:], 0.0)
                dvallw = stp.tile([P, NRT, 32], F32, tag="dvall", name=f"dvall{u}")
                cm32 = stp.tile([P, 32], F32, tag="cm32", name=f"cm32{u}")
                return (racc, dvallw, cm32)

            def m_phase_jg(u, pt, ag_out, st, jg):
                """One jg group of M^T = (S12_hat @ S21_hat)^T[jtile, R_c].

                fp8 DoubleRow matmuls: contraction over pairs of 128-k-blocks.
                PSUM holds QS^2 * M; rescaled by 1/QS^2 on the PSUM->SBUF copy.

                Diag handling per jtile j:
                  tmp4 = msb * imask4           (candidate diag at (p, 128b+p))
                  dv4[p,b] = rowsum of block b  (= msb[p,128b+p])
                  dv4w = dv4 * (-w[j,b])        (w one-hot over (j,b), host input)
                  dvallw[:, :, j] = dv4w        (for the diag output)
                  msb += imask4 * bcast(-BIG*w[j,b])  (kills the true diag only)
                """
                racc, dvallw, cm32 = st
                nwselJ = nwsel.rearrange("p (j b) -> p j b", b=NRT)
                nwselBJ = nwselBIG.rearrange("p (j b) -> p j b", b=NRT)
                if True:
                    pss = [psB.tile([P, 512], F32, tag="ps", name=f"mps{u}_{jg}_{j2}")
                           for j2 in range(4)]
                    for kb4 in range(NKB // 4):
                        qsb = qsbp.tile([P, 4, 512], F8, tag="qsb",
                                        name=f"qs{u}_{jg}_{kb4}")
                        nc.sync.dma_start(
                            qsb[:], ag_out[kb4 * 4 * P:(kb4 + 1) * 4 * P,
                                           jg * 512:(jg + 1) * 512]
                            .rearrange("(o p) n -> p o n", p=P))
                        for h in range(2):
                            for j2 in range(4):
                                nc.tensor.matmul(
                                    pss[j2][:],
                                    qsb[:, 2 * h:2 * h + 2, j2 * P:(j2 + 1) * P],
                                    pt[:, 4 * kb4 + 2 * h:4 * kb4 + 2 * h + 2, :],
                                    start=(kb4 == 0 and h == 0),
                                    stop=(kb4 == NKB // 4 - 1 and h == 1),
                                    perf_mode=DR)
                    for j2 in range(4):
                        j = 4 * jg + j2
                        msb = stp.tile([P, 512], F32, tag="msb", name=f"msb{u}_{j}")
                        nc.scalar.activation(msb[:], pss[j2][:], ACT.Copy,
                                             bias=0.0, scale=1.0 / (QS * QS))
                        # diag candidates (for the diag output, off critical path)
                        tmp4 = stp.tile([P, 512], F32, tag="tmp4", name=f"t4_{u}_{j}")
                        nc.gpsimd.tensor_tensor(tmp4[:], msb[:], imask4[:], op=OP.mult)
                        # kill the true diag with -BIG (independent of the reduce)
                        sc = stp.tile([P, NRT, P], F32, tag="tmp4", name=f"sc_{u}_{j}")
                        nc.gpsimd.tensor_tensor(
                            sc[:], imask4[:],
                            nwselBJ[:, j, :, None].to_broadcast((P, NRT, P)),
                            op=OP.mult)
                        nc.gpsimd.tensor_add(
                            msb.rearrange("p (b q) -> p b q", q=P), msb.rearrange(
                                "p (b q) -> p b q", q=P), sc[:])
                        nc.vector.reduce_max(cm32[:, j:j + 1], msb[:], axis=AX.X)
                        nc.vector.tensor_tensor(racc[:], racc[:], msb[:], op=OP.max)
                        dv4 = smp.tile([P, NRT], F32, tag="sm4", name=f"dv4_{u}_{j}")
                        nc.vector.reduce_sum(
                            dv4[:], tmp4.rearrange("p (b q) -> p b q", q=P), axis=AX.X)
                        dv4w = smp.tile([P, NRT], F32, tag="sm4", name=f"dvw_{u}_{j}")
                        nc.gpsimd.tensor_tensor(dv4w[:], dv4[:], nwselJ[:, j, :],
                                                op=OP.mult)
                        nc.gpsimd.tensor_copy(dvallw[:, :, j], dv4w[:])
            def m_phase_end(u, st):
                racc, dvallw, cm32 = st
                # diag output = -sum_j dvallw
                dsum = smp.tile([P, NRT], F32, tag="sm4", name=f"dsum{u}")
                nc.vector.reduce_sum(dsum[:], dvallw[:], axis=AX.X)
                diag = smp.tile([P, NRT], F32, tag="sm4", name=f"diag{u}")
                nc.vector.tensor_scalar_mul(diag[:], dsum[:], -1.0)
                nc.sync.dma_start(out[u, :, 0:RPC], racc[:])
                nc.sync.dma_start(out[u, :, RPC:RPC + 32], cm32[:])
                nc.sync.dma_start(out[u, :, RPC + 32:OUT_W], diag[:])

            # ---------------- main pipeline ----------------
            prev = None   # (u, pt, ag_out)
            for t, (is_tri, gk, la, ra, lat, rat) in enumerate(TERMS):
                if is_tri:
                    lhs_at = compute_ut(gk, lat, f"at{t}")
                else:
                    lhs_at = load_x(lat, f"at{t}")
                ag_out = at_phase(t, lhs_at, rat)
                if t == 0:
                    # gram partials fill the PE while term 0's softmax/cast
                    # backlog drains; their AllReduce queues after AG0.
                    compute_gram_partials()
                    kick_gram_ar(2)
                if 0 < t < 3:
                    kick_gram_ar(2 - t)
                if is_tri:
                    lhs_a = compute_ut(gk, la, f"a{t}")
                else:
                    lhs_a = load_x(la, f"a{t}")
                # interleave prev term's M-phase jg groups with this term's
                # A-side strips: each fills the other's DMA/drain stalls.
                if prev is not None:
                    mstate = m_phase_start(prev[0])
                    mcb = (lambda s, pv=prev, ms=mstate:
                           m_phase_jg(pv[0], pv[1], pv[2], ms, s))
                else:
                    mstate, mcb = None, None
                pt = a_phase(t, lhs_a, ra, cb=mcb)
                if prev is not None:
                    m_phase_end(prev[0], mstate)
                prev = (t, pt, ag_out)
            mstate = m_phase_start(prev[0])
            for jg in range(NJG):
                m_phase_jg(prev[0], prev[1], prev[2], mstate, jg)
            m_phase_end(prev[0], mstate)

    nc.finalize()
    return nc


_PROGRAM = None


def _get_program():
    global _PROGRAM
    if _PROGRAM is None:
        _PROGRAM = build_program()
    return _PROGRAM


def _normalize(x):
    n = np.linalg.norm(x.astype(np.float32), axis=-1, keepdims=True)
    return (x / np.maximum(n, 1e-12)).astype(np.float32)


def _build_in_maps(inputs):
    nf = [_normalize(np.asarray(inputs[k], np.float32))
          for k in ("feat0", "feat1", "feat2")]
    nfb = [x.astype(ml_dtypes.bfloat16) for x in nf]
    nfT = [np.ascontiguousarray(x.T) for x in nfb]

    in_maps = []
    for c in range(NC):
        rows = slice(c * RPC, (c + 1) * RPC)
        m = {}
        for i in range(3):
            m[f"x{i}"] = np.ascontiguousarray(nfT[i][:, rows])
            m[f"w{i}"] = np.ascontiguousarray(nfb[i][rows])
            m[f"f{i}"] = nfT[i]
        wsel = np.zeros((P, P), np.float32)
        for b in range(NRT):
            j = 4 * c + b
            wsel[:, 4 * j + b] = 1.0     # wselJ[p, j, b] layout
        m["wsel"] = wsel
        in_maps.append(m)
    return in_maps


def _reduce(results):
    """results: list (per core) of {'out': [6, 128, OUT_W]} -> scalar loss."""
    L = np.zeros(6, np.float64)
    for t in range(6):
        rowpart = 0.0
        colmax = np.full(NTOK, -np.inf)
        diag_g = np.zeros(NTOK)
        for c in range(NC):
            o = results[c]["out"][t].astype(np.float64)
            racc = o[:, 0:RPC]
            cm32 = o[:, RPC:RPC + 32]
            dacc = o[:, RPC + 32:OUT_W]
            rowmax_local = racc.max(axis=0)                   # [512]
            diag_local = dacc.T.reshape(RPC)                  # [512]
            rowpart += np.maximum(rowmax_local + MARGIN - diag_local, 0.0).sum()
            colmax = np.maximum(colmax, cm32.T.reshape(NTOK))
            diag_g[c * RPC:(c + 1) * RPC] = diag_local
        colpart = np.maximum(colmax + MARGIN - diag_g, 0.0).sum()
        L[t] = (rowpart + colpart) / (2.0 * NTOK)
    loss = (L[0] + L[1] + L[2]) / 3.0 + (L[3] + L[4] + L[5]) / 3.0
    return np.float32(loss)


def kernel(feat0, feat1, feat2):
    in_maps = _build_in_maps({"feat0": feat0, "feat1": feat1, "feat2": feat2})
    nc = _get_program()
    res = run_bass_kernel_spmd(nc, in_maps, core_ids=list(range(NC)))
    return _reduce(res.results)


if __name__ == "__main__":
    rng = np.random.default_rng(0)
    f0 = rng.standard_normal((NTOK, D), dtype=np.float32)
    f1 = rng.standard_normal((NTOK, D), dtype=np.float32)
    f2 = rng.standard_normal((NTOK, D), dtype=np.float32)
    print("loss:", kernel(f0, f1, f2))


# revision 16
# speedup vs baseline: 1.0066x; 1.0066x over previous
"""Trainium2 Bass kernel for the pairwise+triplewise cycle-consistency loss.

Strategy (8 NeuronCores, tensor-parallel over rows of each [N,N] block):
  - All six cycle-term matrices have the form  A = U @ nf_j^T  with
    U = nf_i (pairs) or U = nf_i @ G_k (triples), G_k = nf_k^T nf_k [D,D].
    This collapses the [N,N]@[N,N] triple products into [D,D] Gram matmuls.
  - Each core owns a 512-row block R_c. Per term it computes A[R_c,:] and
    A^T[R_c,:] directly (bf16 matmuls, f32 PSUM), row-softmaxes both,
    AllGathers the normalized S21_hat (fp8e4, x64 scaled), and computes M^T
    column-tiles (S12_hat @ S21_hat)^T[jtile, R_c] with fp8 DoubleRow
    matmuls, accumulating rowmax/colmax/diag stats on the fly. The scalar
    loss is assembled on host.
  - Grams are computed as per-core partials and AllReduced in bf16. The
    gram work is placed after term 0's AllGather kick so the AllReduces
    don't delay the first AllGather on the single CC stream.
"""
import sys
sys.path.insert(0, "/opt/trn_rl_repo")

import math
import numpy as np
import ml_dtypes

import concourse.bass as bass
import concourse.mybir as mybir
import concourse.tile as tile
from concourse import bacc
from concourse.bass_utils import run_bass_kernel_spmd
from concourse.masks import make_identity

F32 = mybir.dt.float32
BF16 = mybir.dt.bfloat16
F8 = mybir.dt.float8e4
AX = mybir.AxisListType
OP = mybir.AluOpType
ACT = mybir.ActivationFunctionType
DR = mybir.MatmulPerfMode.DoubleRow

NTOK = 4096          # rows per view
D = 1024             # feature dim
NC = 8               # cores
RPC = NTOK // NC     # rows per core (512)
P = 128
NRT = RPC // P       # rowtiles per core (4)
NS = 8               # 512-col strips of A
DKB = D // P         # d-blocks (8)
NKB = NTOK // P      # k-blocks of the M product (32)
NJG = 8              # jtile groups (4 jtiles each) in M product
SCALE = math.log(NTOK) / 0.1
MARGIN = 0.5
QS = 64.0            # fp8 quant scale for softmax probs (M scaled by QS*QS)

# term table: (is_tri, gram_idx, lhsA, rhsA, lhsAT, rhsAT); lhs names index x_i,
# rhs names index f_i. For tri terms lhs is G[gram_idx] @ x_i.
TERMS = [
    (False, None, 0, 1, 1, 0),   # S01
    (False, None, 0, 2, 2, 0),   # S02
    (False, None, 1, 2, 2, 1),   # S12
    (True, 2, 0, 1, 1, 0),       # S02 @ S21 = nf0 G2 nf1^T
    (True, 1, 0, 2, 2, 0),       # S01 @ S12 = nf0 G1 nf2^T
    (True, 0, 1, 2, 2, 1),       # S10 @ S02 = nf1 G0 nf2^T
]

OUT_W = RPC + 32 + NRT   # racc 512 | colmax32 32 | diag 4


def build_program():
    nc = bacc.Bacc("TRN2", target_bir_lowering=False, debug=False, num_devices=NC)

    xs = [nc.dram_tensor(f"x{i}", [D, RPC], BF16, kind="ExternalInput") for i in range(3)]
    ws = [nc.dram_tensor(f"w{i}", [RPC, D], BF16, kind="ExternalInput") for i in range(3)]
    fs = [nc.dram_tensor(f"f{i}", [D, NTOK], BF16, kind="ExternalInput") for i in range(3)]
    wsel_in = nc.dram_tensor("wsel", [P, P], F32, kind="ExternalInput")
    out = nc.dram_tensor("out", [6, P, OUT_W], F32, kind="ExternalOutput")

    with tile.TileContext(nc) as tc:
        with (
            tc.tile_pool(name="cst", bufs=1) as cst,
            tc.tile_pool(name="lhs", bufs=2) as lhsp,
            tc.tile_pool(name="rhs", bufs=2) as rhsp,
            tc.tile_pool(name="aq", bufs=6) as aqp,
            tc.tile_pool(name="pt", bufs=1) as ptp,
            tc.tile_pool(name="qbf", bufs=2) as qbfp,
            tc.tile_pool(name="qsb", bufs=4) as qsbp,
            tc.tile_pool(name="st", bufs=3) as stp,
            tc.tile_pool(name="sm", bufs=6) as smp,
            tc.tile_pool(name="psB", bufs=6, space="PSUM") as psB,
            tc.tile_pool(name="psT", bufs=2, space="PSUM") as psT,
            tc.tile_pool(name="dram", bufs=1, space="DRAM") as dram,
            tc.tile_pool(name="dram2", bufs=2, space="DRAM") as dram2,
        ):
            # constants
            ident = cst.tile([P, P], F32)
            make_identity(nc, ident)
            identb = cst.tile([P, P], BF16)
            nc.vector.tensor_copy(identb[:], ident[:])
            wsel = cst.tile([P, P], F32)
            nc.sync.dma_start(wsel[:], wsel_in[:])
            nwsel = cst.tile([P, P], F32)
            nc.vector.tensor_scalar_mul(nwsel[:], wsel[:], -1.0)
            nwselBIG = cst.tile([P, P], F32)
            nc.vector.tensor_scalar_mul(nwselBIG[:], wsel[:], -1.0e30)
            # imask4[p, 128b+p] = 1 for b in 0..3 (diag candidate positions)
            imask4 = cst.tile([P, NRT, P], F32)
            for b in range(NRT):
                nc.vector.tensor_copy(imask4[:, b, :], ident[:])

            gins = [dram.tile([D, D], BF16, tag=f"gin{k}", name=f"gin{k}")
                    for k in range(3)]
            gouts = [dram.tile([D, D], BF16, tag=f"gout{k}", addr_space="Shared",
                               name=f"gout{k}") for k in range(3)]

            def compute_gram_partials():
                for k in range(3):
                    w_sb = aqp.tile([P, NRT, D], BF16, tag="aq", name=f"w_sb{k}")
                    nc.sync.dma_start(w_sb[:], ws[k].rearrange("(o p) d -> p o d", p=P))
                    for d1 in range(DKB):
                        for d2 in range(2):
                            ps = psB.tile([P, 512], F32, tag="ps", name=f"gps{k}_{d1}_{d2}")
                            for nt in range(NRT):
                                nc.tensor.matmul(
                                    ps[:], w_sb[:, nt, d1 * P:(d1 + 1) * P],
                                    w_sb[:, nt, d2 * 512:(d2 + 1) * 512],
                                    start=(nt == 0), stop=(nt == NRT - 1))
                            gtmp = stp.tile([P, 512], BF16, tag="gtmp", name=f"gt{k}_{d1}_{d2}")
                            nc.vector.tensor_copy(gtmp[:], ps[:])
                            nc.sync.dma_start(
                                gins[k][d1 * P:(d1 + 1) * P,
                                        d2 * 512:(d2 + 1) * 512], gtmp[:])

            def kick_gram_ar(k):
                nc.gpsimd.collective_compute(
                    "AllReduce", OP.add, replica_groups=[list(range(NC))],
                    ins=[gins[k][:]], outs=[gouts[k][:]])

            # ---------------- helpers ----------------
            def load_x(i, nm):
                t = lhsp.tile([P, DKB, RPC], BF16, tag="lhs", name=f"x_{nm}")
                nc.sync.dma_start(t[:], xs[i].rearrange("(o p) r -> p o r", p=P))
                return t

            def compute_ut(gk, i, nm):
                """U^T[:, R_c] = G_k @ x_i  -> [128, DKB, RPC] bf16 tile."""
                x_sb = load_x(i, f"utx_{nm}")
                ut = lhsp.tile([P, DKB, RPC], BF16, tag="lhs", name=f"ut_{nm}")
                for grp in range(2):
                    pss = [psB.tile([P, 512], F32, tag="ps", name=f"utps_{nm}_{grp}_{d4}")
                           for d4 in range(4)]
                    for half in range(2):
                        gh = rhsp.tile([P, 4, D], BF16, tag="rhs", name=f"gh_{nm}_{grp}_{half}")
                        nc.sync.dma_start(
                            gh[:], gouts[gk][half * 512:(half + 1) * 512]
                            .rearrange("(o p) d -> p o d", p=P))
                        for d4 in range(4):
                            dp = 4 * grp + d4
                            for db in range(4):
                                nc.tensor.matmul(
                                    pss[d4][:], gh[:, db, dp * P:(dp + 1) * P],
                                    x_sb[:, 4 * half + db, :],
                                    start=(half == 0 and db == 0),
                                    stop=(half == 1 and db == 3))
                    for d4 in range(4):
                        nc.scalar.copy(ut[:, 4 * grp + d4, :], pss[d4][:])
                return ut

            def side_chunk(lhs_t, fj, nm, cb=None):
                """A-side chunk [RPC, 4096] raw logits (pre-scale) in 4 quarter tiles.

                cb(s), if given, is invoked after each 512-col strip's matmuls;
                used to interleave the previous term's M-phase jg groups."""
                chunk = [aqp.tile([P, NTOK], F32, tag="aq", name=f"ch_{nm}_{rt}")
                         for rt in range(NRT)]
                for s in range(NS):
                    rsb = rhsp.tile([P, DKB, 512], BF16, tag="rhs", name=f"rs_{nm}_{s}")
                    nc.sync.dma_start(
                        rsb[:], fs[fj][:, s * 512:(s + 1) * 512]
                        .rearrange("(o p) n -> p o n", p=P))
                    for rt in range(NRT):
                        ps = psB.tile([P, 512], F32, tag="ps", name=f"aps_{nm}_{s}_{rt}")
                        for kb in range(DKB):
                            nc.tensor.matmul(
                                ps[:], lhs_t[:, kb, rt * P:(rt + 1) * P],
                                rsb[:, kb, :], start=(kb == 0), stop=(kb == DKB - 1))
                        nc.scalar.copy(chunk[rt][:, s * 512:(s + 1) * 512], ps[:])
                    if cb is not None:
                        cb(s)
                return chunk

            def softmax_quarter(q, nm):
                """in-place exp(SCALE*(x - rowmax)); returns QS/rowsum [P,1]."""
                rm = smp.tile([P, 1], F32, tag="sm", name=f"rm_{nm}")
                nc.vector.reduce_max(rm[:], q[:], axis=AX.X)
                bias = smp.tile([P, 1], F32, tag="sm", name=f"bias_{nm}")
                nc.vector.tensor_scalar_mul(bias[:], rm[:], -SCALE)
                ssum = smp.tile([P, 1], F32, tag="sm", name=f"ss_{nm}")
                nc.scalar.activation(q[:], q[:], ACT.Exp, bias=bias[:], scale=SCALE,
                                     accum_out=ssum[:])
                rs = smp.tile([P, 1], F32, tag="sm", name=f"rs_{nm}")
                nc.vector.reciprocal(rs[:], ssum[:])
                rsq = smp.tile([P, 1], F32, tag="sm", name=f"rsq_{nm}")
                nc.vector.tensor_scalar_mul(rsq[:], rs[:], QS)
                return rsq

            def at_phase(t, lhs_t, fj):
                """A^T side: softmax rows -> normalized fp8 (xQS) -> allgather kick."""
                ag_in = dram2.tile([RPC, NTOK], F8, tag="agin", name=f"agin{t}")
                ag_out = dram2.tile([NTOK, NTOK], F8, tag="agout",
                                    addr_space="Shared", name=f"agout{t}")
                chunk = side_chunk(lhs_t, fj, f"at{t}")
                for rt in range(NRT):
                    rq = softmax_quarter(chunk[rt], f"at{t}_{rt}")
                    qb = qbfp.tile([P, NTOK], F8, tag="qbf", name=f"qb{t}_{rt}")
                    nc.vector.tensor_scalar_mul(qb[:], chunk[rt][:], rq[:])
                    nc.sync.dma_start(ag_in[rt * P:(rt + 1) * P, :], qb[:])
                nc.gpsimd.collective_compute(
                    "AllGather", OP.bypass, replica_groups=[list(range(NC))],
                    ins=[ag_in[:]], outs=[ag_out[:]])
                return ag_out

            def a_phase(t, lhs_t, fj, cb=None):
                """A side: softmax, normalize to fp8 (xQS), transpose into PT tile."""
                pt = ptp.tile([P, NKB, RPC], F8, tag="pt", name=f"pt{t}")
                chunk = side_chunk(lhs_t, fj, f"a{t}", cb=cb)
                for rt in range(NRT):
                    rp = softmax_quarter(chunk[rt], f"a{t}_{rt}")
                    qa = qbfp.tile([P, NTOK], BF16, tag="qbf", name=f"qa{t}_{rt}")
                    nc.vector.tensor_scalar_mul(qa[:], chunk[rt][:], rp[:])
                    for j in range(NKB):
                        tp = psT.tile([P, P], BF16, tag="psT", name=f"tp{t}_{rt}_{j}")
                        nc.tensor.transpose(tp[:], qa[:, j * P:(j + 1) * P],
                                            identb[:])
                        nc.vector.tensor_copy(pt[:, j, rt * P:(rt + 1) * P], tp[:])
                return pt

            def m_phase_start(u):
                """Alloc/zero the stat accumulators for term u's M-phase."""
                racc = stp.tile([P, RPC], F32, tag="racc", name=f"racc{u}")
                nc.vector.memset(racc[:], 0.0)
                dvallw = stp.tile([P, NRT, 32], F32, tag="dvall", name=f"dvall{u}")
                cm32 = stp.tile([P, 32], F32, tag="cm32", name=f"cm32{u}")
                return (racc, dvallw, cm32)

            def m_phase_jg(u, pt, ag_out, st, jg):
                """One jg group of M^T = (S12_hat @ S21_hat)^T[jtile, R_c].

                fp8 DoubleRow matmuls: contraction over pairs of 128-k-blocks.
                PSUM holds QS^2 * M; rescaled by 1/QS^2 on the PSUM->SBUF copy.

                Diag handling per jtile j:
                  tmp4 = msb * imask4           (candidate diag at (p, 128b+p))
                  dv4[p,b] = rowsum of block b  (= msb[p,128b+p])
                  dv4w = dv4 * (-w[j,b])        (w one-hot over (j,b), host input)
                  dvallw[:, :, j] = dv4w        (for the diag output)
                  msb += imask4 * bcast(-BIG*w[j,b])  (kills the true diag only)
                """
                racc, dvallw, cm32 = st
                nwselJ = nwsel.rearrange("p (j b) -> p j b", b=NRT)
                nwselBJ = nwselBIG.rearrange("p (j b) -> p j b", b=NRT)
                if True:
                    pss = [psB.tile([P, 512], F32, tag="ps", name=f"mps{u}_{jg}_{j2}")
                           for j2 in range(4)]
                    for kb4 in range(NKB // 4):
                        qsb = qsbp.tile([P, 4, 512], F8, tag="qsb",
                                        name=f"qs{u}_{jg}_{kb4}")
                        nc.sync.dma_start(
                            qsb[:], ag_out[kb4 * 4 * P:(kb4 + 1) * 4 * P,
                                           jg * 512:(jg + 1) * 512]
                            .rearrange("(o p) n -> p o n", p=P))
                        for h in range(2):
                            for j2 in range(4):
                                nc.tensor.matmul(
                                    pss[j2][:],
                                    qsb[:, 2 * h:2 * h + 2, j2 * P:(j2 + 1) * P],
                                    pt[:, 4 * kb4 + 2 * h:4 * kb4 + 2 * h + 2, :],
                                    start=(kb4 == 0 and h == 0),
                                    stop=(kb4 == NKB // 4 - 1 and h == 1),
                                    perf_mode=DR)
                    for j2 in range(4):
                        j = 4 * jg + j2
                        msb = stp.tile([P, 512], F32, tag="msb", name=f"msb{u}_{j}")
                        nc.scalar.activation(msb[:], pss[j2][:], ACT.Copy,
                                             bias=0.0, scale=1.0 / (QS * QS))
                        # diag candidates (for the diag output, off critical path)
                        tmp4 = stp.tile([P, 512], F32, tag="tmp4", name=f"t4_{u}_{j}")
                        nc.gpsimd.tensor_tensor(tmp4[:], msb[:], imask4[:], op=OP.mult)
                        # kill the true diag with -BIG (independent of the reduce)
                        sc = stp.tile([P, NRT, P], F32, tag="tmp4", name=f"sc_{u}_{j}")
                        nc.gpsimd.tensor_tensor(
                            sc[:], imask4[:],
                            nwselBJ[:, j, :, None].to_broadcast((P, NRT, P)),
                            op=OP.mult)
                        nc.gpsimd.tensor_add(
                            msb.rearrange("p (b q) -> p b q", q=P), msb.rearrange(
                                "p (b q) -> p b q", q=P), sc[:])
                        nc.vector.reduce_max(cm32[:, j:j + 1], msb[:], axis=AX.X)
                        nc.vector.tensor_tensor(racc[:], racc[:], msb[:], op=OP.max)
                        dv4 = smp.tile([P, NRT], F32, tag="sm4", name=f"dv4_{u}_{j}")
                        nc.vector.reduce_sum(
                            dv4[:], tmp4.rearrange("p (b q) -> p b q", q=P), axis=AX.X)
                        dv4w = smp.tile([P, NRT], F32, tag="sm4", name=f"dvw_{u}_{j}")
                        nc.gpsimd.tensor_tensor(dv4w[:], dv4[:], nwselJ[:, j, :],
                                                op=OP.mult)
                        nc.gpsimd.tensor_copy(dvallw[:, :, j], dv4w[:])
            def m_phase_end(u, st):
                racc, dvallw, cm32 = st
                # diag output = -sum_j dvallw
                dsum = smp.tile([P, NRT], F32, tag="sm4", name=f"dsum{u}")
                nc.vector.reduce_sum(dsum[:], dvallw[:], axis=AX.X)
                diag = smp.tile([P, NRT], F32, tag="sm4", name=f"diag{u}")
                nc.vector.tensor_scalar_mul(diag[:], dsum[:], -1.0)
                nc.sync.dma_start(out[u, :, 0:RPC], racc[:])
                nc.sync.dma_start(out[u, :, RPC:RPC + 32], cm32[:])
                nc.sync.dma_start(out[u, :, RPC + 32:OUT_W], diag[:])

            # ---------------- main pipeline ----------------
            prev = None   # (u, pt, ag_out)
            for t, (is_tri, gk, la, ra, lat, rat) in enumerate(TERMS):
                if is_tri:
                    lhs_at = compute_ut(gk, lat, f"at{t}")
                else:
                    lhs_at = load_x(lat, f"at{t}")
                ag_out = at_phase(t, lhs_at, rat)
                if 0 < t < 3:
                    kick_gram_ar(2 - t)
                if is_tri:
                    lhs_a = compute_ut(gk, la, f"a{t}")
                else:
                    lhs_a = load_x(la, f"a{t}")
                # interleave prev term's M-phase jg groups with this term's
                # A-side strips: each fills the other's DMA/drain stalls.
                if prev is not None:
                    mstate = m_phase_start(prev[0])
                    mcb = (lambda s, pv=prev, ms=mstate:
                           m_phase_jg(pv[0], pv[1], pv[2], ms, s))
                else:
                    mstate, mcb = None, None
                pt = a_phase(t, lhs_a, ra, cb=mcb)
                if prev is not None:
                    m_phase_end(prev[0], mstate)
                if t == 0:
                    # gram partials after a_phase(0): late enough that their
                    # AllReduce lands behind AG0 on the single CC stream.
                    compute_gram_partials()
                    kick_gram_ar(2)
                prev = (t, pt, ag_out)
            mstate = m_phase_start(prev[0])
            for jg in range(NJG):
                m_phase_jg(prev[0], prev[1], prev[2], mstate, jg)
            m_phase_end(prev[0], mstate)

    nc.finalize()
    return nc


_PROGRAM = None


def _get_program():
    global _PROGRAM
    if _PROGRAM is None:
        _PROGRAM = build_program()
    return _PROGRAM


def _normalize(x):
    n = np.linalg.norm(x.astype(np.float32), axis=-1, keepdims=True)
    return (x / np.maximum(n, 1e-12)).astype(np.float32)


def _build_in_maps(inputs):
    nf = [_normalize(np.asarray(inputs[k], np.float32))
          for k in ("feat0", "feat1", "feat2")]
    nfb = [x.astype(ml_dtypes.bfloat16) for x in nf]
    nfT = [np.ascontiguousarray(x.T) for x in nfb]

    in_maps = []
    for c in range(NC):
        rows = slice(c * RPC, (c + 1) * RPC)
        m = {}
        for i in range(3):
            m[f"x{i}"] = np.ascontiguousarray(nfT[i][:, rows])
            m[f"w{i}"] = np.ascontiguousarray(nfb[i][rows])
            m[f"f{i}"] = nfT[i]
        wsel = np.zeros((P, P), np.float32)
        for b in range(NRT):
            j = 4 * c + b
            wsel[:, 4 * j + b] = 1.0     # wselJ[p, j, b] layout
        m["wsel"] = wsel
        in_maps.append(m)
    return in_maps


def _reduce(results):
    """results: list (per core) of {'out': [6, 128, OUT_W]} -> scalar loss."""
    L = np.zeros(6, np.float64)
    for t in range(6):
        rowpart = 0.0
        colmax = np.full(NTOK, -np.inf)
        diag_g = np.zeros(NTOK)
        for c in range(NC):
            o = results[c]["out"][t].astype(np.float64)
            racc = o[:, 0:RPC]
            cm32 = o[:, RPC:RPC + 32]
            dacc = o[:, RPC + 32:OUT_W]
            rowmax_local = racc.max(axis=0)                   # [512]
            diag_local = dacc.T.reshape(RPC)                  # [512]
            rowpart += np.maximum(rowmax_local + MARGIN - diag_local, 0.0).sum()
            colmax = np.maximum(colmax, cm32.T.reshape(NTOK))
            diag_g[c * RPC:(c + 1) * RPC] = diag_local
        colpart = np.maximum(colmax + MARGIN - diag_g, 0.0).sum()
        L[t] = (rowpart + colpart) / (2.0 * NTOK)
    loss = (L[0] + L[1] + L[2]) / 3.0 + (L[3] + L[4] + L[5]) / 3.0
    return np.float32(loss)


def kernel(feat0, feat1, feat2):
    in_maps = _build_in_maps({"feat0": feat0, "feat1": feat1, "feat2": feat2})
    nc = _get_program()
    res = run_bass_kernel_spmd(nc, in_maps, core_ids=list(range(NC)))
    return _reduce(res.results)


if __name__ == "__main__":
    rng = np.random.default_rng(0)
    f0 = rng.standard_normal((NTOK, D), dtype=np.float32)
    f1 = rng.standard_normal((NTOK, D), dtype=np.float32)
    f2 = rng.standard_normal((NTOK, D), dtype=np.float32)
    print("loss:", kernel(f0, f1, f2))
